# revision 12
# baseline (speedup 1.0000x reference)
"""Trainium2 Bass kernel for the nn_Circuit recurrence.

Math: a 7-state nonlinear EMA circuit scanned over T=2,000,000 steps:
    pv'  = 0.25*relu(Wffpv@stim + Wlat@pyr) + 0.75*pv
    pyr' = 0.1 *relu(Wffy @stim - Wiy@pv' + Wfby@hva) + 0.9*pyr
    hva' = 0.1 *relu(Wffh @pyr') + 0.9*hva
The recurrence forgets exponentially (empirical contraction ~0.94/step), so
the sequence is split into S = NCORES*P*F independent streams, each warmed up
for W steps from a mean-state init using the true preceding inputs.

The whole per-step chain runs on ONE engine (gpsimd/Pool): same-engine
dependencies are enforced by program order, so no cross-engine semaphore
traffic and no per-instruction sync-wait pressure.  Pool has no
scalar_tensor_tensor, so the EMA updates are made plain tensor_tensor adds by
keeping the state PRE-SCALED with a geometric growth that absorbs the decay:
within a renorm block of B steps, the state entering local step e is stored as
    P^ = pyr/0.9^e,  H^ = (hva_{k-1}/h_scale)/0.9^e,  X^ = (c_q*pv)/0.75^e
and every decay multiply folds into the relu's dual-scalar tensor_scalar or
into host-prescaled inputs.  Every B steps two tensor_scalar ops renormalize
the state back to e=0.  The host rescales recorded outputs per step.

Input per step (host precomputed): At'' = (Wffpv@stim)/(wlat*0.9^e) and
Bt'' = A_PYR*(Wffy@stim)/0.75^(e+1).
"""

import os as _os

import numpy as np

T_TOTAL = 2_000_000
NCORES = 8
P = 128

A_PV = np.float32(0.25)
A_PYR = np.float32(0.1)

MASK_FFY = np.array(
    [[1, 1, 0, 0, 0, 0], [0, 0, 1, 1, 0, 0], [0, 0, 0, 0, 1, 1]], np.float32
)
MASK_IY = np.array([[1, 0], [1, 1], [0, 1]], np.float32)
MASK_FFPV = np.array([[1, 1, 1, 0, 0, 0], [0, 0, 0, 1, 1, 1]], np.float32)
MASK_LAT = np.array([[1, 1, 0], [0, 1, 1]], np.float32)
MASK_FFH = np.ones((2, 3), np.float32)
MASK_FBY = np.ones((3, 2), np.float32)

# tunables
F = 2        # streams per partition (total S = NCORES*P*F)
WARM = 96    # warmup steps per stream (mean-init; contraction ~0.94/step)
NH = 1       # H state columns
NB = 32      # renorm block: state stored pre-scaled by 0.9^-e / 0.75^-e
NIN = 4      # input dma chunks (in+out DMAs must fit the 8 HWDGE lanes)
NOUT = 4     # output dma segments (tapered: small final tail)

NS = 5 + NH  # state slots per stream: [P0,P1,P2,H,X0,X1]

# state means for warm-start init (measured steady-state of the circuit)
MEAN_PYR = (0.613, 0.473, 0.602)
MEAN_PV = (0.815, 0.806)
MEAN_HVA = 1.687


def _patch_tile_drain():
    """This walrus build accepts at most ONE sync wait per instruction, but
    Tile's kernel-tail drain waits on every active proc at once.  Split it
    into a chain of single-wait drain instructions (SP executes in order, so
    the chain is semantically identical)."""
    import concourse.mybir as mybir
    from concourse import tile as _tile
    from concourse.vector_clock import ScopedClock

    if getattr(_tile.TileContext, "_drain_split_patched", False):
        return

    def _drain_and_barrier(self, tick_clock, wait_clock):
        drain_inst = self.nc.sync.drain()
        wait_clock.add_sem_waits(
            drain_inst.ins, ScopedClock({None: tick_clock.global_clock})
        )
        si = drain_inst.ins.sync_info
        if si is not None and si.on_wait and len(si.on_wait) > 1:
            waits = list(si.on_wait)
            upds = list(si.on_update) if si.on_update else []
            drain_inst.ins.sync_info = mybir.SyncInfo(
                on_wait=[waits[0]], on_update=[]
            )
            for w in waits[1:-1]:
                d = self.nc.sync.drain()
                d.ins.sync_info = mybir.SyncInfo(on_wait=[w], on_update=[])
            d = self.nc.sync.drain()
            d.ins.sync_info = mybir.SyncInfo(on_wait=[waits[-1]], on_update=upds)
        self.nc.all_engine_barrier()
        popped = self.nc._tile_sem_poison_stack.pop()
        assert popped is self._sem_poison
        self.nc.clear_and_free_semaphores(list(self.sems.allocated().values()))
        self.nc.all_engine_barrier()

    _tile.TileContext._drain_and_barrier = _drain_and_barrier
    _tile.TileContext._drain_split_patched = True


def _sc(e, c_lv, c_fb):
    """Per-local-step compile-time unit-conversion scalars."""
    cx = float(c_lv) * 0.9 ** e / 0.75 ** (e + 1)
    ch = 1.0 / 0.9
    chb = float(c_fb) * (0.9 / 0.75) ** (e + 1)
    cp = (0.75 / 0.9) ** (e + 1)
    return cx, ch, chb, cp


def _build_nc(F, W, L, c_lv, c_fb, nh):
    import concourse.bass as bass
    import concourse.mybir as mybir
    from contextlib import ExitStack
    from concourse.tile import TileContext

    _patch_tile_drain()

    AL = mybir.AluOpType
    f32 = mybir.dt.float32
    steps = W + L
    NSl = 5 + nh
    SW = NSl * F

    nc = bass.Bass(trn_type="TRN2", use_seq_codegen=True)
    X = nc.dram_tensor("x", [P, SW + steps * 5 * F], f32, kind="ExternalInput")
    Y = nc.dram_tensor("y", [P, L * SW], f32, kind="ExternalOutput")

    with ExitStack() as ctx:
        tc = ctx.enter_context(TileContext(nc))
        spool = ctx.enter_context(tc.tile_pool(name="state", bufs=1))
        ST = spool.tile([P, L * SW], f32, name="ST")
        RS = spool.tile([P, 2 * SW], f32, name="RS")   # warmup ping-pong
        RNR = spool.tile([P, SW], f32, name="RNR")     # renormed state slot
        S2t = spool.tile([P, 2 * F], f32, name="S2t")
        GS = spool.tile([P, 3 * F], f32, name="GS")    # [G(2F) | S3(F)]
        RX = spool.tile([P, 2 * F], f32, name="RX")
        RHt = spool.tile([P, F], f32, name="RHt")
        HB = spool.tile([P, 3 * F], f32, name="HB")
        V = spool.tile([P, 3 * F], f32, name="V")
        U3 = spool.tile([P, 3 * F], f32, name="U3")
        RP = spool.tile([P, 3 * F], f32, name="RP")
        ipool = ctx.enter_context(tc.tile_pool(name="inp", bufs=1))

        g = nc.gpsimd

        # input DMAs: NIN resident chunks with geometrically growing sizes so
        # every chunk's (parallel) transfer finishes before compute reaches
        # it.  Chunk 0 is tiny and also carries the SW-wide init state block.
        # DMA transfers serialize; chunk c must fully land before compute
        # (45 ns/step) reaches it while transfers supply ~15.4 ns/step, so
        # boundaries can grow at most ~2.9x per chunk.  Use 2.3x + slack.
        sizes = []
        b = 0
        n = 12
        while b + n < steps and len(sizes) < NIN - 1:
            sizes.append(n)
            b += n
            n = min(int(1.3 * b + 100), steps - b)
        sizes.append(steps - b)
        sizes = [n for n in sizes if n > 0]
        bounds = [0]
        for n in sizes:
            bounds.append(bounds[-1] + n)
        in_tiles = []
        for c, n in enumerate(sizes):
            pad = SW if c == 0 else 0
            t = ipool.tile([P, pad + n * 5 * F], f32, name=f"inchunk{c}")
            lo = 0 if c == 0 else SW + bounds[c] * 5 * F
            hi = SW + bounds[c + 1] * 5 * F
            nc.sync.dma_start(out=t[:, :], in_=X[:, lo:hi])
            in_tiles.append(t)

        def chunk_of(k):
            for c in range(len(sizes)):
                if k < bounds[c + 1]:
                    return in_tiles[c], (k - bounds[c]) * 5 * F + (SW if c == 0 else 0)
            raise AssertionError

        def slot(k):
            # state location after step k (k = -1 is the DMA'd init block)
            if k < 0:
                return in_tiles[0][:, 0:SW]
            if k < W:
                o = (k % 2) * SW
                return RS[:, o : o + SW]
            j = k - W
            return ST[:, j * SW : (j + 1) * SW]

        # output segment boundaries (in output-step space), tapered so the
        # final segment (the only serial tail) is small
        fr = [0.0, 0.32, 0.62, 0.97, 1.0][: NOUT + 1]
        oseg = sorted({round(f * L) for f in fr})
        oseg_i = 0

        for k in range(steps):
            e = k % NB
            cx, ch, chb, cp = _sc(e, c_lv, c_fb)
            prev = RNR[:, :] if (k > 0 and e == 0) else slot(k - 1)
            cur = slot(k)
            it, off = chunk_of(k)
            At = it[:, off : off + 2 * F]
            Bt = it[:, off + 2 * F : off + 5 * F]
            btv = Bt.rearrange("p (c f) -> p c f", c=3)

            # P-sums of prev pyr: S2 = [P0+P1, P1+P2]; S3 = S2a + P2
            g.tensor_tensor(S2t[:, :], prev[:, 0 : 2 * F], prev[:, F : 3 * F], AL.add)
            g.tensor_tensor(GS[:, 0 : 2 * F], S2t[:, :], At, AL.add)
            g.tensor_tensor(
                GS[:, 2 * F : 3 * F], S2t[:, 0:F], prev[:, 2 * F : 3 * F], AL.add
            )
            # relus with unit conversion (dual-scalar tensor_scalar)
            g.tensor_scalar(RX[:, :], GS[:, 0 : 2 * F], 0.0, cx, AL.max, AL.mult)
            g.tensor_scalar(RHt[:, :], GS[:, 2 * F : 3 * F], 0.0, ch, AL.max, AL.mult)
            # EMAs as plain adds (pre-scaled state)
            g.tensor_tensor(
                cur[:, 4 * F : 6 * F], prev[:, 4 * F : 6 * F], RX[:, :], AL.add
            )  # Xv'
            g.tensor_tensor(
                cur[:, 3 * F : 4 * F], prev[:, 3 * F : 4 * F], RHt[:, :], AL.add
            )  # H'
            # HB = chb * H' broadcast over the 3 pyr rows
            hb = (
                cur[:, 3 * F : 4 * F]
                .rearrange("p (a f) -> p a f", a=1)
                .to_broadcast([P, 3, F])
            )
            g.tensor_scalar(
                HB[:, :].rearrange("p (c f) -> p c f", c=3), hb, chb, None, AL.mult
            )
            # V = Bt'' - [X0', X0'+X1', X1']
            vv = V[:, :].rearrange("p (c f) -> p c f", c=3)
            g.tensor_tensor(
                vv[:, 0::2, :],
                btv[:, 0::2, :],
                cur[:, 4 * F : 6 * F].rearrange("p (c f) -> p c f", c=2),
                AL.subtract,
            )
            g.tensor_tensor(
                V[:, F : 2 * F], cur[:, 4 * F : 5 * F], cur[:, 5 * F : 6 * F], AL.add
            )
            g.tensor_tensor(V[:, F : 2 * F], btv[:, 1, :], V[:, F : 2 * F], AL.subtract)
            # U3, relu with unit conversion, P' EMA
            g.tensor_tensor(U3[:, :], V[:, :], HB[:, :], AL.add)
            g.tensor_scalar(RP[:, :], U3[:, :], 0.0, cp, AL.max, AL.mult)
            g.tensor_tensor(cur[:, 0 : 3 * F], prev[:, 0 : 3 * F], RP[:, :], AL.add)

            # renorm every NB steps: back to local exponent 0
            if (k + 1) % NB == 0 and k + 1 < steps:
                g.tensor_scalar(
                    RNR[:, 0 : 4 * F], cur[:, 0 : 4 * F], 0.9**NB, None, AL.mult
                )
                g.tensor_scalar(
                    RNR[:, 4 * F : 6 * F], cur[:, 4 * F : 6 * F], 0.75**NB, None, AL.mult
                )

            # stream finished output segments out while the loop continues
            if k >= W and oseg_i < len(oseg) - 1 and (k - W + 1) == oseg[oseg_i + 1]:
                lo, hi = oseg[oseg_i], oseg[oseg_i + 1]
                if not _os.environ.get("K_NO_OUT_DMA"):
                    nc.sync.dma_start(
                        out=Y[:, lo * SW : hi * SW], in_=ST[:, lo * SW : hi * SW]
                    )
                oseg_i += 1

    return nc


def _prep_inputs(I, Wffpv, Wffy, wlat, W, L, F):
    """Per-core DRAM input arrays (P, SW + steps*5F), fp32, laid out
    [init(SW)] [step][At0,At1,Bt0,Bt1,Bt2][lane], with the per-step renorm
    pre-scaling folded in."""
    S = NCORES * P * F
    steps = W + L
    SW = NS * F
    Aff = I @ Wffpv.T.astype(np.float32)          # (T,2)
    Bff = (I @ Wffy.T.astype(np.float32)) * A_PYR  # (T,3)
    FF = np.concatenate([Aff, Bff], axis=1).astype(np.float32)  # (T,5)

    FFp = np.zeros((W + S * L, 5), np.float32)
    FFp[W : W + T_TOTAL] = FF
    sv = np.lib.stride_tricks.as_strided(
        FFp,
        shape=(S, steps, 5),
        strides=(L * FFp.strides[0], FFp.strides[0], FFp.strides[1]),
    )
    arr = sv.copy()  # (S, steps, 5)
    # fold per-step unit scales: At'' = Aff/(wlat*0.9^e); Bt'' = Bt/0.75^(e+1)
    e = np.arange(steps) % NB
    arr[:, :, 0:2] /= (np.float32(wlat) * 0.9**e)[None, :, None].astype(np.float32)
    arr[:, :, 2:5] /= (0.75 ** (e + 1))[None, :, None].astype(np.float32)

    # stream s = (core*P + p)*F + j  ->  core-local (P, steps, 5, F)
    arr = arr.reshape(NCORES, P, F, steps, 5).transpose(0, 1, 3, 4, 2)
    arr = np.ascontiguousarray(arr).reshape(NCORES, P, steps * 5 * F)

    # init block: mean state (true units, e=0), stream 0 starts from zeros;
    # the unit-dependent H/Xv lanes are overwritten by the caller.
    init = np.empty((NCORES, P, NS, F), np.float32)
    init[..., 0, :] = MEAN_PYR[0]
    init[..., 1, :] = MEAN_PYR[1]
    init[..., 2, :] = MEAN_PYR[2]
    init[..., 3, :] = MEAN_HVA  # overwritten by caller (unit-dependent)
    init[..., 4, :] = MEAN_PV[0]  # overwritten by caller
    init[..., 5, :] = MEAN_PV[1]
    return arr, init.reshape(NCORES, P, NS * F)


def _assemble_output(outs, c_q, h_scale, W, L, F):
    """outs: per-core (P, L*SW) pre-scaled recorded states -> (7, T)."""
    SW = NS * F
    Y = np.stack(outs)  # (NCORES, P, L*SW)
    Y = Y.reshape(NCORES, P, L, NS, F)
    # undo the renorm pre-scaling: output of step k recorded at exponent
    # x = (global step index k+W... careful: loop index) -- loop step k_loop =
    # W + j for output j, exponent after step = (k_loop % NB) + 1
    j = np.arange(L)
    x = ((W + j) % NB) + 1
    s9 = (0.9**x).astype(np.float32)
    s75 = (0.75**x).astype(np.float32)
    Y = Y.transpose(0, 1, 4, 2, 3).reshape(NCORES * P * F, L, NS)
    res7 = np.empty((NCORES * P * F, L, 7), np.float32)
    res7[:, :, 0:3] = Y[:, :, 0:3] * s9[None, :, None]
    res7[:, :, 3:5] = Y[:, :, 4:6] * (s75 / np.float32(c_q))[None, :, None]
    res7[:, :, 5] = Y[:, :, 3] * (s9 * np.float32(h_scale))[None, :]
    res7[:, :, 6] = res7[:, :, 5]
    return np.ascontiguousarray(res7.reshape(-1, 7)[:T_TOTAL].T)


def _mask_weights(W_FFpv, W_LatPV, W_FFy, W_Iy, W_FFh, W_FBy):
    return (
        np.maximum(np.asarray(W_FFpv, np.float32), 0) * MASK_FFPV,
        np.maximum(np.asarray(W_LatPV, np.float32), 0) * MASK_LAT,
        np.maximum(np.asarray(W_FFy, np.float32), 0) * MASK_FFY,
        np.maximum(np.asarray(W_Iy, np.float32), 0) * MASK_IY,
        np.maximum(np.asarray(W_FFh, np.float32), 0) * MASK_FFH,
        np.maximum(np.asarray(W_FBy, np.float32), 0) * MASK_FBY,
    )


def _uniform(vals):
    vals = np.asarray(vals)
    return vals.size > 0 and np.all(vals == vals.flat[0])


def _numpy_fallback(I, Wffpv, Wlat, Wffy, Wiy, Wffh, Wfby, W=1024):
    """General (non-uniform-weight) streamed scan, numpy only."""
    S = 4096
    L = (T_TOTAL + S - 1) // S
    steps = W + L
    Aff = (I @ Wffpv.T).astype(np.float32)
    Bff = (I @ Wffy.T).astype(np.float32)
    FF = np.concatenate([Aff, Bff], axis=1)
    FFp = np.zeros((W + S * L, 5), np.float32)
    FFp[W : W + T_TOTAL] = FF
    sv = np.lib.stride_tricks.as_strided(
        FFp,
        shape=(S, steps, 5),
        strides=(L * FFp.strides[0], FFp.strides[0], FFp.strides[1]),
    )
    Xs = np.ascontiguousarray(sv)
    pyr = np.zeros((S, 3), np.float32)
    pv = np.zeros((S, 2), np.float32)
    hva = np.zeros((S, 2), np.float32)
    out = np.zeros((S, L, 7), np.float32)
    WlatT = Wlat.T.astype(np.float32)
    WiyT = Wiy.T.astype(np.float32)
    WffhT = Wffh.T.astype(np.float32)
    WfbyT = Wfby.T.astype(np.float32)
    for k in range(steps):
        a = Xs[:, k, 0:2]
        b = Xs[:, k, 2:5]
        pv = A_PV * np.maximum(a + pyr @ WlatT, 0) + (1 - A_PV) * pv
        pyr_n = (
            A_PYR * np.maximum(b - pv @ WiyT + hva @ WfbyT, 0) + (1 - A_PYR) * pyr
        )
        hva_n = A_PYR * np.maximum(pyr_n @ WffhT, 0) + (1 - A_PYR) * hva
        if k >= W:
            out[:, k - W, 0:3] = pyr_n
            out[:, k - W, 3:5] = pv
            out[:, k - W, 5:7] = hva
        pyr, hva = pyr_n, hva_n
    return np.ascontiguousarray(out.reshape(S * L, 7)[:T_TOTAL].T)


def kernel(I, W_FFpv, W_LatPV, W_FFy, W_Iy, W_FFh, W_FBy):
    I = np.asarray(I, np.float32)
    Wffpv, Wlat, Wffy, Wiy, Wffh, Wfby = _mask_weights(
        W_FFpv, W_LatPV, W_FFy, W_Iy, W_FFh, W_FBy
    )

    wlat = Wlat[0, 0]
    wiy = Wiy[0, 0]
    wffh = Wffh[0, 0]
    wfby = Wfby[0, 0]
    fast = (
        _uniform(Wlat[MASK_LAT > 0])
        and _uniform(Wiy[MASK_IY > 0])
        and _uniform(Wffh)
        and _uniform(Wfby)
        and wffh > 0
        and wiy > 0
        and wlat > 0
    )
    if not fast:
        return _numpy_fallback(I, Wffpv, Wlat, Wffy, Wiy, Wffh, Wfby)

    c_q = np.float32(A_PYR * wiy)       # Xv = c_q * pv
    h_scale = np.float32(A_PYR * wffh)  # hva = h_scale * H (delayed)
    c_lv = np.float32(c_q * A_PV * wlat)
    c_fb = np.float32(A_PYR * wfby * 2.0 * h_scale)

    S = NCORES * P * F
    L = (T_TOTAL + S - 1) // S

    try:
        from concourse.bass_utils import run_bass_kernel_spmd

        nc = _build_nc(F, WARM, L, float(c_lv), float(c_fb), NH)
        arr, init = _prep_inputs(I, Wffpv, Wffy, wlat, WARM, L, F)
        # init block in true pre-scaled units (e=0): [P0,P1,P2,H,X0,X1]
        init = init.reshape(NCORES, P, NS, F)
        init[..., 3, :] = np.float32(MEAN_HVA / h_scale)
        init[..., 4, :] = np.float32(c_q * MEAN_PV[0])
        init[..., 5, :] = np.float32(c_q * MEAN_PV[1])
        init[0, 0, :, 0] = 0.0  # stream 0 = true zero start
        xs = [
            np.concatenate(
                [init[c].reshape(P, NS * F), arr[c]], axis=1
            ).astype(np.float32)
            for c in range(NCORES)
        ]
        res = run_bass_kernel_spmd(
            nc, [{"x": x} for x in xs], core_ids=list(range(NCORES))
        )
        outs = [res.results[c]["y"] for c in range(NCORES)]
        return _assemble_output(outs, c_q, h_scale, WARM, L, F)
    except Exception:
        return _numpy_fallback(I, Wffpv, Wlat, Wffy, Wiy, Wffh, Wfby)
